# revision 11
# baseline (speedup 1.0000x reference)
"""Trainium2 Bass kernel for the Performer-HMM language model forward pass.

Math: exp(project_logits(a,b))[i,j] = Phi_a[i] . Phi_b[j] exactly, where
Phi_f = exp(f @ proj - 0.5||f||^2) (row/global stabilized).  Hence
  transition_exp = diag(1/r) Phix Phiy^T        (rank D=512),
  emission probs b_t[c]   = (Phit[c].g_t)/e_den[c],  g_t = Phiv[token_t],
and the scaled forward recursion in linear space is
  u_t = ((u_{t-1} * w) @ Phix) @ Phiy^T * b_t,   O_t = log(sum u_t) - log(sum u_{t-1})
with w = 1/r.  The CxC transition / CxV emission are never materialized.

Device layout: C=8192 sharded 8 ways (1024 states/core, 8 chunks of 128
partitions).  State kept transposed [c-chunk, 16].  Per step:
  mm2: uT[c,n] (8x4 matmuls)  -> vT = uT*BWT_t  -> mm1: zT[d,n] + c-row
  (8x5 matmuls, c-row via an extra r-column of Phix)  -> AllReduce of the
  [128,80] z/c tile across 8 cores -> next step.  Evidence row recorded per
  step; host takes logs and differences (deferred normalization, renorm
  every RENORM_K steps on device).
"""

import numpy as np
from contextlib import ExitStack

C, H, D, V, N, T = 8192, 256, 512, 10000, 16, 256
NCORES = 8
CL = C // NCORES           # 1024 states per core
NCH = CL // 128            # 8 chunks of 128 partitions
KD = D // 128              # 4 d-chunks
RENORM_K = 4
TN = T * N                 # 4096


# ----------------------------------------------------------------------------
# Host-side precompute (cheap O(C*H*D) projections; heavy work stays on device)
# ----------------------------------------------------------------------------

def _host_precompute(params, text):
    f32 = np.float32
    p = {k: np.asarray(v, f32) for k, v in params.items()}

    def res(x, w1, b1, w2, b2):
        h = np.maximum(x @ w1 + b1, 0.0)
        return x + np.maximum(h @ w2 + b2, 0.0)

    proj = p['proj']

    def logphi(f):
        return f @ proj - 0.5 * np.sum(f * f, -1, keepdims=True)

    fx0 = p['start_emb'][None] @ p['sm_w'] + p['sm_b']
    fx0 = res(fx0, p['s1_w1'], p['s1_b1'], p['s1_w2'], p['s1_b2'])
    fx0 = res(fx0, p['s2_w1'], p['s2_b1'], p['s2_w2'], p['s2_b2'])
    l0 = logphi(fx0)
    Phi0 = np.exp(l0 - l0.max())                       # [1, D]

    ly = logphi(p['next_state_emb'])
    Phiy = np.exp(ly - ly.max())                       # [C, D] global stab
    sy = Phiy.sum(0)                                   # [D]
    s_prob = (Phi0 @ Phiy.T)[0] / (Phi0 @ sy)[0]       # [C]

    lx = logphi(p['state_emb'])
    Phix = np.exp(lx - lx.max(-1, keepdims=True))      # [C, D] row stab
    r = Phix @ sy                                      # [C]
    w = 1.0 / r

    ft = res(p['preterminal_emb'], p['t1_w1'], p['t1_b1'], p['t1_w2'], p['t1_b2'])
    ft = res(ft, p['t2_w1'], p['t2_b1'], p['t2_w2'], p['t2_b2'])
    lt = logphi(ft)
    Phit = np.exp(lt - lt.max(-1, keepdims=True))      # [C, D] row stab

    lv = logphi(p['terminal_emb'])
    Phiv = np.exp(lv - lv.max())                       # [V, D] global stab
    sv = Phiv.sum(0)
    e_den = Phit @ sv                                  # [C]
    wden = w / e_den                                   # [C]

    G = Phiv[np.asarray(text)]                         # [N, T, D] host gather
    Gt = np.ascontiguousarray(G.transpose(2, 1, 0))    # [D, T, N]

    Phix_aug = np.concatenate([Phix, r[:, None]], axis=1)  # [C, 513]
    return Phix_aug, Phiy, Phit, Gt, s_prob, wden


def _per_core_arrays(Phix_aug, Phiy, Phit, Gt, s_prob, wden):
    import ml_dtypes
    bf16 = ml_dtypes.bfloat16
    cores = []
    gt = np.ascontiguousarray(
        Gt.reshape(KD, 128, T, N).transpose(1, 0, 2, 3).reshape(128, KD * TN)
    ).astype(bf16)
    for k in range(NCORES):
        sl = slice(k * CL, (k + 1) * CL)
        pa = Phix_aug[sl].reshape(NCH, 128, 513).transpose(1, 0, 2).reshape(128, NCH * 513)
        py = Phiy[sl].reshape(NCH, 128, KD, 128).transpose(3, 2, 0, 1).reshape(128, KD * CL)
        pt = Phit[sl].reshape(NCH, 128, KD, 128).transpose(3, 2, 0, 1).reshape(128, KD * CL)
        st = np.ascontiguousarray(s_prob[sl].reshape(NCH, 128).T)
        wd = np.ascontiguousarray(wden[sl].reshape(NCH, 128).T)
        cores.append({
            "phixa": np.ascontiguousarray(pa).astype(bf16),
            "phiyt": np.ascontiguousarray(py).astype(bf16),
            "phitt": np.ascontiguousarray(pt).astype(bf16),
            "gt": gt,
            "st": st.astype(np.float32),
            "wden": wd.astype(np.float32),
        })
    return cores


# ----------------------------------------------------------------------------
# Device kernel
# ----------------------------------------------------------------------------

def _build_module(T_steps=T):
    from concourse import bacc, tile, mybir
    from concourse import bass

    f32 = mybir.dt.float32
    bf = mybir.dt.bfloat16
    nc = bacc.Bacc(trn_type="TRN2", target_bir_lowering=False, num_devices=NCORES)

    p_phixa = nc.declare_dram_parameter("phixa", [128, NCH * 513], bf, isOutput=False)
    p_phiyt = nc.declare_dram_parameter("phiyt", [128, KD * CL], bf, isOutput=False)
    p_phitt = nc.declare_dram_parameter("phitt", [128, KD * CL], bf, isOutput=False)
    p_gt = nc.declare_dram_parameter("gt", [128, KD * TN], bf, isOutput=False)
    p_st = nc.declare_dram_parameter("st", [128, NCH], f32, isOutput=False)
    p_wden = nc.declare_dram_parameter("wden", [128, NCH], f32, isOutput=False)
    p_chat = nc.declare_dram_parameter("chat", [1, T_steps * N], f32, isOutput=True)

    cc_in = nc.dram_tensor("cc_in", [128, 80], f32)
    cc_out = nc.dram_tensor("cc_out", [128, 80], f32, addr_space="Shared")
    groups = [list(range(NCORES))]

    with tile.TileContext(nc) as tc:
        with tc.tile_pool(name="main", bufs=1) as pool:
            # persistent SBUF tensors
            phixa = pool.tile([128, NCH * 513], bf)
            phiyt = pool.tile([128, KD * CL], bf)
            phitt = pool.tile([128, KD * CL], bf)
            gt = pool.tile([128, KD * TN], bf)
            st = pool.tile([128, NCH], f32)
            wden = pool.tile([128, NCH], f32)
            bwt = pool.tile([128, NCH * TN], bf)       # emission*w, [c-chunk, t, n]
            vt = pool.tile([128, NCH * N], bf)         # vT state
            ztb = pool.tile([128, KD * N], bf)         # reduced z, bf16
            chat = pool.tile([1, T_steps * N], f32)
            rc = pool.tile([1, N], f32)                # reciprocal of c-row
            sends = [pool.tile([128, 80], f32, name=f"send{i}") for i in range(2)]
            recvs = [pool.tile([128, 80], f32, name=f"recv{i}") for i in range(2)]

            for tl, prm in ((phixa, p_phixa), (phiyt, p_phiyt), (phitt, p_phitt),
                            (gt, p_gt), (st, p_st), (wden, p_wden)):
                nc.sync.dma_start(out=tl[:], in_=prm[:, :])
            for s in sends:
                nc.vector.memset(s[:], 0.0)

            # ---- emission precompute: BWT[c, t, n] = (Phit.g) * w/e_den ----
            with tc.tile_pool(name="pspre", bufs=4, space="PSUM") as psp:
                for m in range(NCH):
                    for f in range(8):                 # 8 groups of 512 cols
                        ps = psp.tile([128, 512], f32, tag="pc")
                        for kd in range(KD):
                            nc.tensor.matmul(
                                ps[:],
                                phitt[:, kd * CL + m * 128: kd * CL + (m + 1) * 128],
                                gt[:, kd * TN + f * 512: kd * TN + (f + 1) * 512],
                                start=(kd == 0), stop=(kd == KD - 1),
                            )
                        nc.scalar.activation(
                            bwt[:, m * TN + f * 512: m * TN + (f + 1) * 512], ps[:],
                            mybir.ActivationFunctionType.Copy,
                            scale=wden[:, m: m + 1],
                        )

            stack = ExitStack()
            psp1 = stack.enter_context(tc.tile_pool(name="ps1", bufs=1, space="PSUM"))
            pspu = stack.enter_context(tc.tile_pool(name="psu", bufs=2, space="PSUM"))
            # one PSUM bank per accumulation group: start=True clears the
            # whole bank, so concurrent groups must not share one.
            z_ps = [psp1.tile([128, N], f32, name=f"zps{i}") for i in range(KD)]
            c_ps = psp1.tile([1, N], f32)              # evidence row
            bc_ps = psp1.tile([128, N], f32)           # 1/c broadcast
            ones = pool.tile([1, 128], f32)
            nc.vector.memset(ones[:], 1.0)

            for t in range(T_steps):
                if t == 0:
                    for m in range(NCH):
                        nc.vector.tensor_scalar_mul(
                            vt[:, m * N: (m + 1) * N],
                            bwt[:, m * TN: m * TN + N],
                            st[:, m: m + 1],
                        )
                else:
                    rv = recvs[(t - 1) % 2]
                    if t % RENORM_K == 0:
                        nc.vector.reciprocal(rc[:], rv[0:1, 64: 64 + N])
                        nc.tensor.matmul(bc_ps[:], ones[:], rc[:],
                                         start=True, stop=True)
                        for kd in range(KD):
                            nc.vector.tensor_tensor(
                                ztb[:, kd * N: (kd + 1) * N],
                                rv[:, kd * N: (kd + 1) * N],
                                bc_ps[:], op=mybir.AluOpType.mult,
                            )
                    else:
                        nc.vector.tensor_copy(ztb[:], rv[:, 0:64])
                    # mm2 + emission multiply
                    for m in range(NCH):
                        u_ps = pspu.tile([128, N], f32, tag="u")
                        for kd in range(KD):
                            nc.tensor.matmul(
                                u_ps[:],
                                phiyt[:, kd * CL + m * 128: kd * CL + (m + 1) * 128],
                                ztb[:, kd * N: (kd + 1) * N],
                                start=(kd == 0), stop=(kd == KD - 1),
                            )
                        nc.vector.tensor_tensor(
                            vt[:, m * N: (m + 1) * N],
                            u_ps[:],
                            bwt[:, m * TN + t * N: m * TN + (t + 1) * N],
                            op=mybir.AluOpType.mult,
                        )
                # mm1: zT (4 chunks) + c-row, one PSUM bank each
                for kc in range(NCH):
                    first, last = kc == 0, kc == NCH - 1
                    for mc in range(KD):
                        nc.tensor.matmul(
                            z_ps[mc][:],
                            phixa[:, kc * 513 + mc * 128: kc * 513 + (mc + 1) * 128],
                            vt[:, kc * N: (kc + 1) * N],
                            start=first, stop=last,
                        )
                    nc.tensor.matmul(
                        c_ps[:],
                        phixa[:, kc * 513 + 512: kc * 513 + 513],
                        vt[:, kc * N: (kc + 1) * N],
                        start=first, stop=last,
                    )
                snd = sends[t % 2]
                for mc in range(KD):
                    nc.vector.tensor_copy(snd[:, mc * N: (mc + 1) * N], z_ps[mc][:])
                nc.vector.tensor_copy(snd[0:1, 64:80], c_ps[:])
                nc.sync.dma_start(out=cc_in[:, :], in_=snd[:])
                nc.gpsimd.collective_compute(
                    "AllReduce", mybir.AluOpType.add,
                    replica_groups=groups, ins=[cc_in[:, :]], outs=[cc_out[:, :]],
                )
                rv = recvs[t % 2]
                nc.sync.dma_start(out=rv[:], in_=cc_out[:, :])
                nc.vector.tensor_copy(chat[0:1, t * N: (t + 1) * N],
                                      rv[0:1, 64: 64 + N])

            nc.sync.dma_start(out=p_chat[:, :], in_=chat[:])
            stack.close()
    nc.compile()
    return nc


# ----------------------------------------------------------------------------
# Entry point
# ----------------------------------------------------------------------------

def kernel(text, mask, params, _profile=None):
    text = np.asarray(text)
    mask = np.asarray(mask)
    Phix_aug, Phiy, Phit, Gt, s_prob, wden = _host_precompute(params, text)
    cores = _per_core_arrays(Phix_aug, Phiy, Phit, Gt, s_prob, wden)

    from concourse import bass_utils
    nc = _build_module(T)
    res = bass_utils.run_bass_kernel_spmd(nc, cores, list(range(NCORES)))
    chat = res.results[0]["chat"].reshape(T, N)        # c-hat per (t, n)
    kernel._last = res

    Ohat = np.log(np.maximum(chat, 1e-38)).T           # [N, T]
    O = Ohat.copy()
    tgrid = np.arange(T)
    nonwin = (tgrid % RENORM_K) != 0
    O[:, nonwin] = Ohat[:, nonwin] - Ohat[:, np.maximum(tgrid - 1, 0)[nonwin]]
    return np.where(mask, O, 0.0).astype(np.float32)
